# revision 4
# baseline (speedup 1.0000x reference)
"""Bass/Tile TRN2 kernel v2 for per-model-batched causal self-attention.

Problem: x[M,B,S,D], qkv_w[M,D,3D], proj_w[M,D,D] -> out[M,B,S,D]
M=8 models sharded across 8 NeuronCores (embarrassingly parallel).

Per-core design (model m), per batch b:
  xb    = bf16(x_b)                       (Pool casts)        [s, d]
  xT    = DMA-XBAR-transpose(xb)          (DMA, no PE)        [d, s] bf16
  xt8   = e4m3(xT)                        (DVE)
  qkT   = DoubleRow fp8 matmul w8^T xt8   (PE @ 2x rate)      [o, s] -> bf16
  V     = xT^T @ wv_bf (bf16)                                 [s, o] -> bf16 + ones col
  st    = K^T Q (bf16, causal-trimmed)    -> bf16 PSUM (1 bank/pair)
  p     = exp(st/(8*1024))  (Act; w8 pre-scaled by 32)
  yT    = V_aug^T @ p  accumulated over k-tiles (bf16)        [hd+1, q] PSUM f32
          row 64 = softmax sums; streams >=128 so no LdWeights exposure
  ynT   = yT[0:64] * pbcast(1/sums)       (recip DVE, bcast Pool, muls DVE/Pool)
  out   = ynT^T @ wproj_bf (bf16)         -> f32, Pool copy, DMA out

Next batch's load/cast/transpose/projection groups are interleaved into the
attention loop (work queue) so the in-order PE has ready work while the
ScalarE exp chain runs.
"""

import sys

if "/opt/trn_rl_repo" not in sys.path:
    sys.path.insert(0, "/opt/trn_rl_repo")

from contextlib import nullcontext
from functools import partial

import numpy as np

import concourse.bass as bass
import concourse.mybir as mybir
import concourse.tile as tile
from concourse import bacc, bass_utils
from concourse.masks import make_upper_triangular

M, B, S, D, H = 8, 4, 512, 512, 8
HD = D // H  # 64
F32 = mybir.dt.float32
BF16 = mybir.dt.bfloat16
E4M3 = mybir.dt.float8e4

N_CORES = 8
W8_SCALE = 32.0

_cache = {}


def build_nc(reps=1):
    nc = bacc.Bacc("TRN2", target_bir_lowering=False, debug=False)

    x_d = nc.dram_tensor("x", [B, S, D], F32, kind="ExternalInput")
    wqkv_d = nc.dram_tensor("wqkv", [D, 3 * D], F32, kind="ExternalInput")
    wproj_d = nc.dram_tensor("wproj", [D, D], F32, kind="ExternalInput")
    out_d = nc.dram_tensor("out", [B, S, D], F32, kind="ExternalOutput")

    with tile.TileContext(nc) as tc:
        with (
            tc.tile_pool(name="singles", bufs=1) as singles,
            tc.tile_pool(name="xp", bufs=2) as xpool,
            tc.tile_pool(name="xbp", bufs=2) as xbpool,
            tc.tile_pool(name="xtp", bufs=2) as xtpool,
            tc.tile_pool(name="x8p", bufs=2) as x8pool,
            tc.tile_pool(name="qk", bufs=2) as qkpool,
            tc.tile_pool(name="vp", bufs=2) as vpool,
            tc.tile_pool(name="se", bufs=2) as sepool,
            tc.tile_pool(name="ytp", bufs=2) as ytpool,
            tc.tile_pool(name="rp", bufs=3) as rpool,
            tc.tile_pool(name="op", bufs=3) as opool,
            tc.tile_pool(name="ps_mm", bufs=2, space=bass.MemorySpace.PSUM) as ps_mm,
            tc.tile_pool(name="ps_big", bufs=3, space=bass.MemorySpace.PSUM) as ps_big,
        ):
          with tc.For_i(0, reps, 1) if reps > 1 else nullcontext():
            # ---- constants ----
            tri2 = singles.tile([128, 2, 128], BF16)  # keep-mask (k<=q), x2 heads
            make_upper_triangular(nc, tri2[:, 0, :], val=1.0, diag=True)
            nc.gpsimd.tensor_copy(out=tri2[:, 1, :], in_=tri2[:, 0, :])

            wqkv = singles.tile([128, 4, 3 * D], F32)
            w8 = singles.tile([128, 4, 1024], E4M3)
            wv_bf = singles.tile([128, 4, 512], BF16)
            wproj_f = singles.tile([128, 4, D], F32)
            wproj = singles.tile([128, 4, D], BF16)

            state = {}

            # ---------- stage A (loads + projections), schedulable groups ----
            def emit_load_x(b):
                x_sb = xpool.tile([128, 4, D], F32, tag="x", name="xsb")
                nc.sync.dma_start(
                    out=x_sb[:],
                    in_=x_d.ap()[b].rearrange("(q p) d -> p q d", p=128),
                )
                v_sb = vpool.tile([128, 4, H, 66], BF16, tag="v", name="vsb")
                nc.gpsimd.memset(v_sb[:, :, :, 64:65], 1.0)
                state[b] = {"x": x_sb, "qkT": {}, "v": v_sb, "ynT": {}}
                if b == 0:
                    # q/k weight columns gate the first matmuls; halves so the
                    # w8 casts can chase
                    for half in range(2):
                        nc.scalar.dma_start(
                            out=wqkv[:, 2 * half : 2 * half + 2, 0:1024],
                            in_=wqkv_d.ap()[
                                half * 256 : half * 256 + 256, 0:1024
                            ].rearrange("(c p) o -> p c o", p=128),
                        )

            def emit_xcast(b, stq):
                st_ = state[b]
                if stq == 0:
                    st_["xb"] = xbpool.tile([128, 4, D], BF16, tag="xb", name="xb")
                nc.gpsimd.tensor_copy(
                    out=st_["xb"][:, stq, :], in_=st_["x"][:, stq, :]
                )

            def emit_xT(b, stq):
                st_ = state[b]
                if stq == 0:
                    st_["xT"] = xtpool.tile([128, 4, D], BF16, tag="xt", name="xt")
                nc.sync.dma_start(
                    out=st_["xT"][:, :, stq * 128 : (stq + 1) * 128],
                    in_=st_["xb"][:, stq, :],
                    transpose=True,
                )

            def emit_xt8(b, half):
                st_ = state[b]
                if half == 0:
                    st_["x8"] = x8pool.tile([128, 4, D], E4M3, tag="x8", name="x8")
                nc.vector.tensor_copy(
                    out=st_["x8"][:, 2 * half : 2 * half + 2, :],
                    in_=st_["xT"][:, 2 * half : 2 * half + 2, :],
                )

            def emit_w8(half):
                # w8 = 32 * wqkv[:, :, :1024] as e4m3 (descaled in exp)
                nc.vector.tensor_scalar_mul(
                    w8[:, 2 * half : 2 * half + 2, :],
                    wqkv[:, 2 * half : 2 * half + 2, 0:1024],
                    W8_SCALE,
                )

            def emit_late_weights():
                nc.scalar.dma_start(
                    out=wqkv[:, :, 1024:1536],
                    in_=wqkv_d.ap()[:, 1024:1536].rearrange("(c p) o -> p c o", p=128),
                )
                nc.scalar.dma_start(
                    out=wproj_f[:],
                    in_=wproj_d.ap().rearrange("(c p) o -> p c o", p=128),
                )
                nc.gpsimd.tensor_copy(out=wv_bf[:], in_=wqkv[:, :, 1024:1536])
                nc.gpsimd.tensor_copy(out=wproj[:], in_=wproj_f[:])

            def emit_qk8_group(b, mt):
                st_ = state[b]
                mp = ps_mm.tile([128, 512], F32, tag="mm", name="mp")
                for g in range(2):
                    nc.tensor.matmul(
                        mp[:],
                        w8[:, 2 * g : 2 * g + 2, mt * 128 : (mt + 1) * 128],
                        st_["x8"][:, 2 * g : 2 * g + 2, :],
                        start=(g == 0),
                        stop=(g == 1),
                        perf_mode=mybir.MatmulPerfMode.DoubleRow,
                    )
                qk = qkpool.tile([128, 512], BF16, tag=f"qk{mt}", name=f"qk{mt}")
                if mt % 2 == 0:
                    nc.vector.tensor_copy(out=qk[:], in_=mp[:])
                else:
                    nc.scalar.copy(out=qk[:], in_=mp[:])
                st_["qkT"][mt] = qk

            def emit_v_group(b, stt):
                st_ = state[b]
                vp_ps = ps_mm.tile([128, 512], F32, tag="mm", name="vp")
                for dc in range(4):
                    nc.tensor.matmul(
                        vp_ps[:],
                        st_["xT"][:, dc, stt * 128 : (stt + 1) * 128],
                        wv_bf[:, dc, :],
                        start=(dc == 0),
                        stop=(dc == 3),
                    )
                nc.scalar.copy(
                    out=st_["v"][:, stt, :, 0:64],
                    in_=vp_ps[:].rearrange("p (h e) -> p h e", h=H),
                )

            def proj_work(b):
                w = [partial(emit_load_x, b)]
                for stq in range(4):
                    w.append(partial(emit_xcast, b, stq))
                    w.append(partial(emit_xT, b, stq))
                w += [partial(emit_xt8, b, 0), partial(emit_xt8, b, 1)]
                if b == 0:
                    w += [partial(emit_w8, 0), partial(emit_w8, 1)]
                    w.append(emit_late_weights)
                w += [
                    partial(emit_qk8_group, b, mt) for mt in (4, 0, 5, 1, 6, 2, 7, 3)
                ]
                w += [partial(emit_v_group, b, stt) for stt in range(4)]
                return w

            # ---------- attention ----------
            def emit_scores(b, hg):
                qkT = state[b]["qkT"]
                se = sepool.tile([128, 4, 2, 512], BF16, tag="se", name="se")
                for kt in range(4):
                    off = 128 * kt
                    stp = ps_big.tile([128, 2, 512], F32, tag="big", name="stp")
                    for hi in range(2):
                        nc.tensor.matmul(
                            stp[:, hi, off:512],
                            qkT[4 + hg][64 * hi : 64 * hi + 64, off : off + 128],
                            qkT[hg][64 * hi : 64 * hi + 64, off:512],
                            start=True,
                            stop=True,
                        )
                    nc.scalar.activation(
                        out=se[:, kt, :, off:],
                        in_=stp[:, :, off:],
                        func=mybir.ActivationFunctionType.Exp,
                        scale=1.0 / (np.sqrt(HD) * W8_SCALE * W8_SCALE),
                    )
                    # mask the diagonal block (strict lower triangle -> 0)
                    nc.gpsimd.tensor_mul(
                        out=se[:, kt, :, off : off + 128],
                        in0=se[:, kt, :, off : off + 128],
                        in1=tri2[:],
                    )
                return se

            def emit_attv_norm(b, hg, se):
                st_ = state[b]
                h0 = 2 * hg
                yp = ps_big.tile([128, 2, 512], F32, tag="big", name="yp")
                for hi in range(2):
                    for kt in range(4):
                        nc.tensor.matmul(
                            yp[0:65, hi, kt * 128 : 512],
                            st_["v"][:, kt, h0 + hi, 0:65],
                            se[:, kt, hi, kt * 128 : 512],
                            start=(kt == 0),
                            stop=(kt == 3),
                        )
                rcp = rpool.tile([1, 2, 512], F32, tag="rcp", name="rcp")
                nc.vector.reciprocal(out=rcp[:], in_=yp[64:65, :, :])
                rcp_b = rpool.tile([64, 2, 512], F32, tag="rcpb", name="rcpb")
                nc.gpsimd.partition_broadcast(rcp_b[:], rcp[:])
                ynT = ytpool.tile([128, 512], BF16, tag=f"yt{hg}", name=f"yt{hg}")
                nc.vector.tensor_mul(
                    out=ynT[0:64, :], in0=yp[0:64, 0, :], in1=rcp_b[:, 0, :]
                )
                nc.vector.tensor_mul(
                    out=ynT[64:128, :], in0=yp[0:64, 1, :], in1=rcp_b[:, 1, :]
                )
                st_["ynT"][hg] = ynT

            def emit_proj_group(b, qt):
                ynT = state[b]["ynT"]
                op_ps = ps_mm.tile([128, 512], F32, tag="mm", name="op")
                for dc in range(4):
                    nc.tensor.matmul(
                        op_ps[:],
                        ynT[dc][:, qt * 128 : (qt + 1) * 128],
                        wproj[:, dc, :],
                        start=(dc == 0),
                        stop=(dc == 3),
                    )
                ob = opool.tile([128, 512], F32, tag="ob", name="ob")
                if qt % 2 == 0:
                    nc.vector.tensor_copy(out=ob[:], in_=op_ps[:])
                else:
                    nc.scalar.copy(out=ob[:], in_=op_ps[:])
                nc.sync.dma_start(
                    out=out_d.ap()[b, qt * 128 : (qt + 1) * 128, :], in_=ob[:]
                )

            # ---------- main schedule ----------
            w0 = proj_work(0)
            for f in w0[:16]:
                f()  # loads, casts, transposes, weights, qkT for head-pair 0
            se_prev = emit_scores(0, 0)
            for f in w0[16:]:
                f()
            pending_proj = []
            for b in range(B):
                queue = (proj_work(b + 1) if b + 1 < B else []) + pending_proj
                for hg in range(4):
                    se_next = emit_scores(b, hg + 1) if hg + 1 < 4 else None
                    # fill PE while ScalarE runs the exp chain for this hg
                    for _ in range(6):
                        if queue:
                            queue.pop(0)()
                    emit_attv_norm(b, hg, se_prev)
                    se_prev = se_next
                while queue:
                    queue.pop(0)()
                # first scores of the next batch fill the normalize tail
                se_prev = emit_scores(b + 1, 0) if b + 1 < B else None
                # this batch's projection is deferred into the next attention
                pending_proj = [partial(emit_proj_group, b, qt) for qt in range(4)]
            for f in pending_proj:
                f()

    nc.compile()
    return nc


def kernel(x, qkv_weight, proj_weight):
    if "nc" not in _cache:
        _cache["nc"] = build_nc()
    nc = _cache["nc"]
    in_maps = [
        {
            "x": np.ascontiguousarray(x[m], dtype=np.float32),
            "wqkv": np.ascontiguousarray(qkv_weight[m], dtype=np.float32),
            "wproj": np.ascontiguousarray(proj_weight[m], dtype=np.float32),
        }
        for m in range(M)
    ]
    res = bass_utils.run_bass_kernel_spmd(nc, in_maps, core_ids=list(range(N_CORES)))
    return np.stack([res.results[m]["out"] for m in range(M)]).astype(np.float32)


# revision 15
# speedup vs baseline: 2.0950x; 2.0950x over previous
"""Bass/Tile TRN2 kernel v2 for per-model-batched causal self-attention.

Problem: x[M,B,S,D], qkv_w[M,D,3D], proj_w[M,D,D] -> out[M,B,S,D]
M=8 models sharded across 8 NeuronCores (embarrassingly parallel).

Per-core design (model m), per batch b:
  xb    = bf16(x_b)                       (Pool casts)        [s, d]
  xT    = DMA-XBAR-transpose(xb)          (DMA, no PE)        [d, s] bf16
  xt8   = e4m3(xT)                        (DVE)
  qkT   = DoubleRow fp8 matmul w8^T xt8   (PE @ 2x rate)      [o, s] -> bf16
  V     = xT^T @ wv_bf (bf16)                                 [s, o] -> bf16 + ones col
  st    = K^T Q (bf16, causal-trimmed)    -> bf16 PSUM (1 bank/pair)
  p     = exp(st/(8*1024))  (Act; w8 pre-scaled by 32)
  yT    = V_aug^T @ p  accumulated over k-tiles (bf16)        [hd+1, q] PSUM f32
          row 64 = softmax sums; streams >=128 so no LdWeights exposure
  ynT   = yT[0:64] * pbcast(1/sums)       (recip DVE, bcast Pool, muls DVE/Pool)
  out   = ynT^T @ wproj_bf (bf16)         -> f32, Pool copy, DMA out

Next batch's load/cast/transpose/projection groups are interleaved into the
attention loop (work queue) so the in-order PE has ready work while the
ScalarE exp chain runs.
"""

import sys

if "/opt/trn_rl_repo" not in sys.path:
    sys.path.insert(0, "/opt/trn_rl_repo")

from contextlib import nullcontext
from functools import partial

import numpy as np

import concourse.bass as bass
import concourse.mybir as mybir
import concourse.tile as tile
from concourse import bacc, bass_utils
from concourse.masks import make_upper_triangular

M, B, S, D, H = 8, 4, 512, 512, 8
HD = D // H  # 64
F32 = mybir.dt.float32
F32R = mybir.dt.float32r
BF16 = mybir.dt.bfloat16
E4M3 = mybir.dt.float8e4

N_CORES = 8
W8_SCALE = 32.0

_cache = {}


def build_nc(reps=1):
    nc = bacc.Bacc("TRN2", target_bir_lowering=False, debug=False)

    x_d = nc.dram_tensor("x", [B, S, D], F32, kind="ExternalInput")
    wqkv_d = nc.dram_tensor("wqkv", [D, 3 * D], F32, kind="ExternalInput")
    wproj_d = nc.dram_tensor("wproj", [D, D], F32, kind="ExternalInput")
    out_d = nc.dram_tensor("out", [B, S, D], F32, kind="ExternalOutput")

    with tile.TileContext(nc) as tc:
        with (
            tc.tile_pool(name="singles", bufs=1) as singles,
            tc.tile_pool(name="xp", bufs=2) as xpool,
            tc.tile_pool(name="xbp", bufs=2) as xbpool,
            tc.tile_pool(name="xtp", bufs=2) as xtpool,
            tc.tile_pool(name="x8p", bufs=2) as x8pool,
            tc.tile_pool(name="qk", bufs=2) as qkpool,
            tc.tile_pool(name="vp", bufs=2) as vpool,
            tc.tile_pool(name="se", bufs=2) as sepool,
            tc.tile_pool(name="ytp", bufs=2) as ytpool,
            tc.tile_pool(name="rp", bufs=3) as rpool,
            tc.tile_pool(name="op", bufs=3) as opool,
            tc.tile_pool(name="ps_mm", bufs=2, space=bass.MemorySpace.PSUM) as ps_mm,
            tc.tile_pool(name="ps_big", bufs=3, space=bass.MemorySpace.PSUM) as ps_big,
        ):
          with tc.For_i(0, reps, 1) if reps > 1 else nullcontext():
            # ---- constants ----
            tri2 = singles.tile([128, 2, 128], BF16)  # keep-mask (k<=q), x2 heads
            make_upper_triangular(nc, tri2[:, 0, :], val=1.0, diag=True)
            nc.vector.tensor_copy(out=tri2[:, 1, :], in_=tri2[:, 0, :])

            wqkv = singles.tile([128, 4, 3 * D], F32)
            w8 = singles.tile([128, 4, 1024], E4M3)
            wv_bf = singles.tile([128, 4, 512], BF16)
            wproj_f = singles.tile([128, 4, D], F32)
            wproj = singles.tile([128, 4, D], BF16)

            state = {}

            # ---------- stage A (loads + projections), schedulable groups ----
            def emit_load_x(b):
                x_sb = xpool.tile([128, 4, D], F32, tag="x", name="xsb")
                nc.sync.dma_start(
                    out=x_sb[:],
                    in_=x_d.ap()[b].rearrange("(q p) d -> p q d", p=128),
                )
                v_sb = vpool.tile([128, 4, H, 66], BF16, tag="v", name="vsb")
                nc.vector.memset(v_sb[:, :, :, 64:65], 1.0)
                state[b] = {"x": x_sb, "qkT": {}, "v": v_sb, "ynT": {}}
                if b == 0:
                    # q/k weight columns gate the first matmuls; halves so the
                    # w8 casts can chase
                    for half in range(2):
                        nc.scalar.dma_start(
                            out=wqkv[:, 2 * half : 2 * half + 2, 0:1024],
                            in_=wqkv_d.ap()[
                                half * 256 : half * 256 + 256, 0:1024
                            ].rearrange("(c p) o -> p c o", p=128),
                        )

            def emit_xcast(b):
                st_ = state[b]
                st_["xb"] = xbpool.tile([128, 4, D], BF16, tag="xb", name="xb")
                nc.scalar.copy(out=st_["xb"][:], in_=st_["x"][:])

            def emit_xT(b, stq):
                st_ = state[b]
                if stq == 0:
                    st_["xT"] = xtpool.tile([128, 4, D], BF16, tag="xt", name="xt")
                nc.sync.dma_start(
                    out=st_["xT"][:, :, stq * 128 : (stq + 1) * 128],
                    in_=st_["xb"][:, stq, :],
                    transpose=True,
                )

            def emit_xt8(b, half):
                st_ = state[b]
                if half == 0:
                    st_["x8"] = x8pool.tile([128, 4, D], E4M3, tag="x8", name="x8")
                nc.vector.tensor_copy(
                    out=st_["x8"][:, 2 * half : 2 * half + 2, :],
                    in_=st_["xT"][:, 2 * half : 2 * half + 2, :],
                )

            def emit_w8(half):
                # w8 = 32 * wqkv[:, :, :1024] as e4m3 (descaled in exp)
                nc.vector.tensor_scalar_mul(
                    w8[:, 2 * half : 2 * half + 2, :],
                    wqkv[:, 2 * half : 2 * half + 2, 0:1024],
                    W8_SCALE,
                )

            def emit_late_weights():
                nc.scalar.dma_start(
                    out=wqkv[:, :, 1024:1536],
                    in_=wqkv_d.ap()[:, 1024:1536].rearrange("(c p) o -> p c o", p=128),
                )
                nc.scalar.dma_start(
                    out=wproj_f[:],
                    in_=wproj_d.ap().rearrange("(c p) o -> p c o", p=128),
                )
                nc.vector.tensor_copy(out=wv_bf[:], in_=wqkv[:, :, 1024:1536])
                nc.scalar.copy(out=wproj[:], in_=wproj_f[:])

            def emit_qk8_group(b, mt):
                st_ = state[b]
                mp = ps_mm.tile([128, 512], F32, tag="mm", name="mp")
                for g in range(2):
                    nc.tensor.matmul(
                        mp[:],
                        w8[:, 2 * g : 2 * g + 2, mt * 128 : (mt + 1) * 128],
                        st_["x8"][:, 2 * g : 2 * g + 2, :],
                        start=(g == 0),
                        stop=(g == 1),
                        perf_mode=mybir.MatmulPerfMode.DoubleRow,
                    )
                qk = qkpool.tile([128, 512], BF16, tag=f"qk{mt}", name=f"qk{mt}")
                nc.vector.tensor_copy(out=qk[:], in_=mp[:])
                st_["qkT"][mt] = qk

            def emit_v_group(b, stt):
                st_ = state[b]
                vp_ps = ps_mm.tile([128, 512], F32, tag="mm", name="vp")
                for dc in range(4):
                    nc.tensor.matmul(
                        vp_ps[:],
                        st_["xT"][:, dc, stt * 128 : (stt + 1) * 128],
                        wv_bf[:, dc, :],
                        start=(dc == 0),
                        stop=(dc == 3),
                    )
                nc.scalar.copy(
                    out=st_["v"][:, stt, :, 0:64],
                    in_=vp_ps[:].rearrange("p (h e) -> p h e", h=H),
                )

            def proj_work(b):
                w = [partial(emit_load_x, b), partial(emit_xcast, b)]
                for stq in range(4):
                    w.append(partial(emit_xT, b, stq))
                w += [partial(emit_xt8, b, 0), partial(emit_xt8, b, 1)]
                if b == 0:
                    w += [partial(emit_w8, 0), partial(emit_w8, 1)]
                    w.append(emit_late_weights)
                w += [
                    partial(emit_qk8_group, b, mt) for mt in (4, 0, 5, 1, 6, 2, 7, 3)
                ]
                w += [partial(emit_v_group, b, stt) for stt in range(4)]
                return w

            # ---------- attention ----------
            def emit_scores(b, hg):
                qkT = state[b]["qkT"]
                se = sepool.tile([128, 4, 2, 512], BF16, tag="se", name="se")
                for kt in range(4):
                    off = 128 * kt
                    stp = ps_big.tile([128, 2, 512], F32, tag="big", name="stp")
                    for hi in range(2):
                        nc.tensor.matmul(
                            stp[:, hi, off:512],
                            qkT[4 + hg][64 * hi : 64 * hi + 64, off : off + 128],
                            qkT[hg][64 * hi : 64 * hi + 64, off:512],
                            start=True,
                            stop=True,
                        )
                    nc.scalar.activation(
                        out=se[:, kt, :, off:],
                        in_=stp[:, :, off:],
                        func=mybir.ActivationFunctionType.Exp,
                        scale=1.0 / (np.sqrt(HD) * W8_SCALE * W8_SCALE),
                    )
                    # mask the diagonal block (strict lower triangle -> 0)
                    nc.vector.tensor_mul(
                        out=se[:, kt, :, off : off + 128],
                        in0=se[:, kt, :, off : off + 128],
                        in1=tri2[:],
                    )
                return se

            def emit_attv_norm(b, hg, se):
                st_ = state[b]
                h0 = 2 * hg
                yp = ps_big.tile([128, 2, 512], F32, tag="big", name="yp")
                for hi in range(2):
                    for kt in range(4):
                        nc.tensor.matmul(
                            yp[0:65, hi, kt * 128 : 512],
                            st_["v"][:, kt, h0 + hi, 0:65],
                            se[:, kt, hi, kt * 128 : 512],
                            start=(kt == 0),
                            stop=(kt == 3),
                        )
                rcp = rpool.tile([1, 2, 512], BF16, tag="rcp", name="rcp")
                with nc.allow_low_precision(reason="bf16 softmax norm factor"):
                    nc.vector.reciprocal(out=rcp[:], in_=yp[64:65, :, :])
                rcp_b = rpool.tile([64, 2, 512], BF16, tag="rcpb", name="rcpb")
                nc.gpsimd.partition_broadcast(rcp_b[:], rcp[:])
                ynT = ytpool.tile([128, 512], BF16, tag=f"yt{hg}", name=f"yt{hg}")
                nc.vector.tensor_mul(
                    out=ynT[0:64, :], in0=yp[0:64, 0, :], in1=rcp_b[:, 0, :]
                )
                nc.vector.tensor_mul(
                    out=ynT[64:128, :], in0=yp[0:64, 1, :], in1=rcp_b[:, 1, :]
                )
                st_["ynT"][hg] = ynT

            def emit_proj_group(b, qt):
                ynT = state[b]["ynT"]
                op_ps = ps_mm.tile([128, 512], F32, tag="mm", name="op")
                for dc in range(4):
                    nc.tensor.matmul(
                        op_ps[:],
                        ynT[dc][:, qt * 128 : (qt + 1) * 128],
                        wproj[:, dc, :],
                        start=(dc == 0),
                        stop=(dc == 3),
                    )
                ob = opool.tile([128, 512], F32, tag="ob", name="ob")
                nc.scalar.copy(out=ob[:], in_=op_ps[:])
                nc.sync.dma_start(
                    out=out_d.ap()[b, qt * 128 : (qt + 1) * 128, :], in_=ob[:]
                )

            # ---------- main schedule ----------
            w0 = proj_work(0)
            for f in w0[:16]:
                f()  # loads, casts, transposes, weights, qkT for head-pair 0
            se_prev = emit_scores(0, 0)
            for f in w0[16:]:
                f()
            pending_proj = []
            for b in range(B):
                queue = (proj_work(b + 1) if b + 1 < B else []) + pending_proj
                for hg in range(4):
                    se_next = emit_scores(b, hg + 1) if hg + 1 < 4 else None
                    # fill PE while ScalarE runs the exp chain for this hg
                    for _ in range(6):
                        if queue:
                            queue.pop(0)()
                    emit_attv_norm(b, hg, se_prev)
                    se_prev = se_next
                while queue:
                    queue.pop(0)()
                # first scores of the next batch fill the normalize tail
                se_prev = emit_scores(b + 1, 0) if b + 1 < B else None
                # this batch's projection is deferred into the next attention
                pending_proj = [partial(emit_proj_group, b, qt) for qt in range(4)]
            for f in pending_proj:
                f()

    nc.compile()
    return nc


def kernel(x, qkv_weight, proj_weight):
    if "nc" not in _cache:
        _cache["nc"] = build_nc()
    nc = _cache["nc"]
    in_maps = [
        {
            "x": np.ascontiguousarray(x[m], dtype=np.float32),
            "wqkv": np.ascontiguousarray(qkv_weight[m], dtype=np.float32),
            "wproj": np.ascontiguousarray(proj_weight[m], dtype=np.float32),
        }
        for m in range(M)
    ]
    res = bass_utils.run_bass_kernel_spmd(nc, in_maps, core_ids=list(range(N_CORES)))
    return np.stack([res.results[m]["out"] for m in range(M)]).astype(np.float32)


# revision 16
# speedup vs baseline: 2.1575x; 1.0298x over previous
"""Bass/Tile TRN2 kernel v11: baseline schedule + bf16 everywhere +
DMA-XBAR transposes for x and y (no PE transposes, no transpose copies).

Per-core design (model m), per batch b:
  xb      = bf16(x_b)              (Act cast)                 [s, d]
  xT      = DMA-XBAR(xb)           (DMA, no PE)               [d, s] bf16
  qkT     = wqk_bf.T @ xT          (bf16 matmul)              [1024, s] -> bf16
  V       = xT.T @ wv_bf           (bf16)                     [s, 512] bf16 + ones col
  st[k,q] = K @ Q^T  (bf16, causal-trimmed)                   PSUM f32
  p       = exp(st/8)  (ScalarE, bf16 out), diag masked by tri01 mul
  y_aug   = p.T @ V_aug (bf16)  -> y[q,d] + sums in col 64    (PSUM)
  y       = y_aug * (1/sums)  per-partition scalar -> bf16
  ynT     = DMA-XBAR(y)  per q-tile at batch end              [d, s] bf16
  out     = ynT.T @ wproj_bf (bf16)

The next batch's load/transpose/projection groups are interleaved into the
attention loop (work queue) so the in-order PE has ready work while the
ScalarE exp chain runs.
"""

import sys

if "/opt/trn_rl_repo" not in sys.path:
    sys.path.insert(0, "/opt/trn_rl_repo")

from contextlib import nullcontext
from functools import partial

import numpy as np

import concourse.bass as bass
import concourse.mybir as mybir
import concourse.tile as tile
from concourse import bacc, bass_utils
from concourse.masks import make_upper_triangular

M, B, S, D, H = 8, 4, 512, 512, 8
HD = D // H  # 64
F32 = mybir.dt.float32
F32R = mybir.dt.float32r
BF16 = mybir.dt.bfloat16

N_CORES = 8

_cache = {}


def build_nc(reps=1):
    nc = bacc.Bacc("TRN2", target_bir_lowering=False, debug=False)

    x_d = nc.dram_tensor("x", [B, S, D], F32, kind="ExternalInput")
    wqkv_d = nc.dram_tensor("wqkv", [D, 3 * D], F32, kind="ExternalInput")
    wproj_d = nc.dram_tensor("wproj", [D, D], F32, kind="ExternalInput")
    out_d = nc.dram_tensor("out", [B, S, D], F32, kind="ExternalOutput")

    with tile.TileContext(nc) as tc:
        with (
            tc.tile_pool(name="singles", bufs=1) as singles,
            tc.tile_pool(name="xp", bufs=2) as xpool,
            tc.tile_pool(name="xbp", bufs=2) as xbpool,
            tc.tile_pool(name="xtp", bufs=2) as xtpool,
            tc.tile_pool(name="qk", bufs=2) as qkpool,
            tc.tile_pool(name="vp", bufs=2) as vpool,
            tc.tile_pool(name="se", bufs=3) as sepool,
            tc.tile_pool(name="yp", bufs=2) as ypool,
            tc.tile_pool(name="ytp", bufs=2) as ytpool,
            tc.tile_pool(name="op", bufs=3) as opool,
            tc.tile_pool(name="rp", bufs=4) as rpool,
            tc.tile_pool(name="ps_mm", bufs=2, space=bass.MemorySpace.PSUM) as ps_mm,
            tc.tile_pool(name="ps_att", bufs=3, space=bass.MemorySpace.PSUM) as ps_att,
        ):
          with tc.For_i(0, reps, 1) if reps > 1 else nullcontext():
            # ---- constants ----
            tri2 = singles.tile([128, 2, 128], BF16)  # keep-mask (k<=q), x2 heads
            make_upper_triangular(nc, tri2[:, 0, :], val=1.0, diag=True)
            nc.vector.tensor_copy(out=tri2[:, 1, :], in_=tri2[:, 0, :])

            wqkv = singles.tile([128, 4, 3 * D], F32)
            wqk_bf = singles.tile([128, 4, 1024], BF16)
            wv_bf = singles.tile([128, 4, 512], BF16)
            wproj_f = singles.tile([128, 4, D], F32)
            wproj = singles.tile([128, 4, D], BF16)

            state = {}

            # ---------- stage A (loads + projections), schedulable groups ----
            def emit_load_x(b):
                x_sb = xpool.tile([128, 4, D], F32, tag="x", name="xsb")
                nc.sync.dma_start(
                    out=x_sb[:],
                    in_=x_d.ap()[b].rearrange("(q p) d -> p q d", p=128),
                )
                v_sb = vpool.tile([128, 4, H, 66], BF16, tag="v", name="vsb")
                nc.vector.memset(v_sb[:, :, :, 64:65], 1.0)
                state[b] = {"x": x_sb, "qkT": {}, "v": v_sb}
                if b == 0:
                    for half in range(2):
                        nc.scalar.dma_start(
                            out=wqkv[:, 2 * half : 2 * half + 2, 0:1024],
                            in_=wqkv_d.ap()[
                                half * 256 : half * 256 + 256, 0:1024
                            ].rearrange("(c p) o -> p c o", p=128),
                        )

            def emit_xcast(b):
                st_ = state[b]
                st_["xb"] = xbpool.tile([128, 4, D], BF16, tag="xb", name="xb")
                nc.scalar.copy(out=st_["xb"][:], in_=st_["x"][:])

            def emit_xT(b, stq):
                st_ = state[b]
                if stq == 0:
                    st_["xT"] = xtpool.tile([128, 4, D], BF16, tag="xt", name="xt")
                nc.sync.dma_start(
                    out=st_["xT"][:, :, stq * 128 : (stq + 1) * 128],
                    in_=st_["xb"][:, stq, :],
                    transpose=True,
                )

            def emit_wqk(half):
                nc.vector.tensor_copy(
                    out=wqk_bf[:, 2 * half : 2 * half + 2, :],
                    in_=wqkv[:, 2 * half : 2 * half + 2, 0:1024],
                )

            def emit_late_weights():
                nc.scalar.dma_start(
                    out=wqkv[:, :, 1024:1536],
                    in_=wqkv_d.ap()[:, 1024:1536].rearrange("(c p) o -> p c o", p=128),
                )
                nc.scalar.dma_start(
                    out=wproj_f[:],
                    in_=wproj_d.ap().rearrange("(c p) o -> p c o", p=128),
                )
                nc.vector.tensor_copy(out=wv_bf[:], in_=wqkv[:, :, 1024:1536])
                nc.scalar.copy(out=wproj[:], in_=wproj_f[:])

            def emit_qkt_group(b, mt):
                st_ = state[b]
                mp = ps_mm.tile([128, 512], F32, tag="mm", name="mp")
                for dc in range(4):
                    nc.tensor.matmul(
                        mp[:],
                        wqk_bf[:, dc, mt * 128 : (mt + 1) * 128],
                        st_["xT"][:, dc, :],
                        start=(dc == 0),
                        stop=(dc == 3),
                    )
                qk = qkpool.tile([128, 512], BF16, tag=f"qk{mt}", name=f"qk{mt}")
                nc.vector.tensor_copy(out=qk[:], in_=mp[:])
                st_["qkT"][mt] = qk

            def emit_v_group(b, stt):
                st_ = state[b]
                vp_ps = ps_mm.tile([128, 512], F32, tag="mm", name="vp")
                for dc in range(4):
                    nc.tensor.matmul(
                        vp_ps[:],
                        st_["xT"][:, dc, stt * 128 : (stt + 1) * 128],
                        wv_bf[:, dc, :],
                        start=(dc == 0),
                        stop=(dc == 3),
                    )
                nc.scalar.copy(
                    out=st_["v"][:, stt, :, 0:64],
                    in_=vp_ps[:].rearrange("p (h e) -> p h e", h=H),
                )

            def proj_work(b):
                w = [partial(emit_load_x, b), partial(emit_xcast, b)]
                w += [partial(emit_xT, b, stq) for stq in range(4)]
                if b == 0:
                    w += [partial(emit_wqk, 0), partial(emit_wqk, 1)]
                    w.append(emit_late_weights)
                w += [partial(emit_qkt_group, b, mt) for mt in (0, 4, 1, 5, 2, 6, 3, 7)]
                w += [partial(emit_v_group, b, stt) for stt in range(4)]
                return w

            # ---------- attention ----------
            def emit_scores(b, hg):
                qkT = state[b]["qkT"]
                h0, h1 = 2 * hg, 2 * hg + 1
                se = sepool.tile([128, 4, 2, 512], BF16, tag="se", name="se")
                for kt in range(4):
                    off = 128 * kt
                    stp = ps_att.tile([128, 1024], F32, tag="att", name="stp")
                    for hi, h in enumerate((h0, h1)):
                        mtq, poq = h // 2, 64 * (h % 2)
                        mtk, pok = 4 + h // 2, 64 * (h % 2)
                        nc.tensor.matmul(
                            stp[:, hi * 512 + off : hi * 512 + 512],
                            qkT[mtk][pok : pok + 64, kt * 128 : (kt + 1) * 128],
                            qkT[mtq][poq : poq + 64, off:512],
                            start=True,
                            stop=True,
                        )
                    nc.scalar.activation(
                        out=se[:, kt, :, off:],
                        in_=stp[:].rearrange("p (hh q) -> p hh q", hh=2)[:, :, off:],
                        func=mybir.ActivationFunctionType.Exp,
                        scale=1.0 / np.sqrt(HD),
                    )
                    # mask the diagonal block (strict lower triangle -> 0)
                    nc.vector.tensor_mul(
                        out=se[:, kt, :, off : off + 128],
                        in0=se[:, kt, :, off : off + 128],
                        in1=tri2[:],
                    )
                return se

            def emit_y(b, hg, se, y_sb):
                st_ = state[b]
                h0, h1 = 2 * hg, 2 * hg + 1
                yp = ps_att.tile([128, 1024], F32, tag="att", name="yp")
                for hi, h in enumerate((h0, h1)):
                    for qt in range(4):
                        base = hi * 512 + qt * 65
                        for kt in range(qt + 1):
                            nc.tensor.matmul(
                                yp[:, base : base + 65],
                                se[:, kt, hi, qt * 128 : (qt + 1) * 128],
                                st_["v"][:, kt, h, 0:65],
                                start=(kt == 0),
                                stop=(kt == qt),
                            )
                rs = rpool.tile([128, 2, 4], F32, tag="rs", name="rs")
                nc.vector.reciprocal_approx_fast(
                    out=rs[:],
                    in_=yp[:].rearrange("p (hh q) -> p hh q", hh=2)[:, :, 64:260:65],
                )
                for hi, h in enumerate((h0, h1)):
                    for qt in range(4):
                        base = hi * 512 + qt * 65
                        nc.vector.tensor_scalar_mul(
                            y_sb[:, qt, 64 * h : 64 * h + 64],
                            yp[:, base : base + 64],
                            rs[:, hi, qt : qt + 1],
                        )

            def emit_yT(b, qt, y_sb, ynT):
                nc.sync.dma_start(
                    out=ynT[:, :, qt * 128 : (qt + 1) * 128],
                    in_=y_sb[:, qt, :],
                    transpose=True,
                )

            def emit_proj_group(b, qt, ynT):
                op_ps = ps_mm.tile([128, 512], F32, tag="mm", name="op")
                for dc in range(4):
                    nc.tensor.matmul(
                        op_ps[:],
                        ynT[:, dc, qt * 128 : (qt + 1) * 128],
                        wproj[:, dc, :],
                        start=(dc == 0),
                        stop=(dc == 3),
                    )
                ob = opool.tile([128, 512], F32, tag="ob", name="ob")
                nc.scalar.copy(out=ob[:], in_=op_ps[:])
                nc.sync.dma_start(
                    out=out_d.ap()[b, qt * 128 : (qt + 1) * 128, :], in_=ob[:]
                )

            # ---------- main schedule ----------
            w0 = proj_work(0)
            for f in w0[:11]:
                f()  # loads, cast, transposes, weights, qkT for head-pair 0
            se_prev = emit_scores(0, 0)
            for f in w0[11:]:
                f()
            pending_proj = []
            for b in range(B):
                queue = (proj_work(b + 1) if b + 1 < B else []) + pending_proj
                y_sb = ypool.tile([128, 4, 512], BF16, tag="y", name="ysb")
                for hg in range(4):
                    se_next = emit_scores(b, hg + 1) if hg + 1 < 4 else None
                    # fill PE while ScalarE runs the exp chain for this hg
                    for _ in range(6):
                        if queue:
                            queue.pop(0)()
                    emit_y(b, hg, se_prev, y_sb)
                    se_prev = se_next
                while queue:
                    queue.pop(0)()
                ynT = ytpool.tile([128, 4, 512], BF16, tag="yt", name="ynT")
                for qt in range(4):
                    emit_yT(b, qt, y_sb, ynT)
                # first scores of the next batch fill the normalize tail
                se_prev = emit_scores(b + 1, 0) if b + 1 < B else None
                # this batch's projection is deferred into the next attention
                pending_proj = [
                    partial(emit_proj_group, b, qt, ynT) for qt in range(4)
                ]
            for f in pending_proj:
                f()

    nc.compile()
    return nc


def kernel(x, qkv_weight, proj_weight):
    if "nc" not in _cache:
        _cache["nc"] = build_nc()
    nc = _cache["nc"]
    in_maps = [
        {
            "x": np.ascontiguousarray(x[m], dtype=np.float32),
            "wqkv": np.ascontiguousarray(qkv_weight[m], dtype=np.float32),
            "wproj": np.ascontiguousarray(proj_weight[m], dtype=np.float32),
        }
        for m in range(M)
    ]
    res = bass_utils.run_bass_kernel_spmd(nc, in_maps, core_ids=list(range(N_CORES)))
    return np.stack([res.results[m]["out"] for m in range(M)]).astype(np.float32)


# revision 17
# speedup vs baseline: 2.1621x; 1.0021x over previous
"""Bass/Tile TRN2 kernel v11: baseline schedule + bf16 everywhere +
DMA-XBAR transposes for x and y (no PE transposes, no transpose copies).

Per-core design (model m), per batch b:
  xb      = bf16(x_b)              (Act cast)                 [s, d]
  xT      = DMA-XBAR(xb)           (DMA, no PE)               [d, s] bf16
  qkT     = wqk_bf.T @ xT          (bf16 matmul)              [1024, s] -> bf16
  V       = xT.T @ wv_bf           (bf16)                     [s, 512] bf16 + ones col
  st[k,q] = K @ Q^T  (bf16, causal-trimmed)                   PSUM f32
  p       = exp(st/8)  (ScalarE, bf16 out), diag masked by tri01 mul
  y_aug   = p.T @ V_aug (bf16)  -> y[q,d] + sums in col 64    (PSUM)
  y       = y_aug * (1/sums)  per-partition scalar -> bf16
  ynT     = DMA-XBAR(y)  per q-tile at batch end              [d, s] bf16
  out     = ynT.T @ wproj_bf (bf16)

The next batch's load/transpose/projection groups are interleaved into the
attention loop (work queue) so the in-order PE has ready work while the
ScalarE exp chain runs.
"""

import sys

if "/opt/trn_rl_repo" not in sys.path:
    sys.path.insert(0, "/opt/trn_rl_repo")

from contextlib import nullcontext
from functools import partial

import numpy as np

import concourse.bass as bass
import concourse.mybir as mybir
import concourse.tile as tile
from concourse import bacc, bass_utils
from concourse.masks import make_upper_triangular

M, B, S, D, H = 8, 4, 512, 512, 8
HD = D // H  # 64
F32 = mybir.dt.float32
F32R = mybir.dt.float32r
BF16 = mybir.dt.bfloat16
E4M3 = mybir.dt.float8e4
W8_SCALE = 32.0

N_CORES = 8

_cache = {}


def build_nc(reps=1):
    nc = bacc.Bacc("TRN2", target_bir_lowering=False, debug=False)

    x_d = nc.dram_tensor("x", [B, S, D], F32, kind="ExternalInput")
    wqkv_d = nc.dram_tensor("wqkv", [D, 3 * D], F32, kind="ExternalInput")
    wproj_d = nc.dram_tensor("wproj", [D, D], F32, kind="ExternalInput")
    out_d = nc.dram_tensor("out", [B, S, D], F32, kind="ExternalOutput")

    with tile.TileContext(nc) as tc:
        with (
            tc.tile_pool(name="singles", bufs=1) as singles,
            tc.tile_pool(name="xp", bufs=2) as xpool,
            tc.tile_pool(name="xbp", bufs=2) as xbpool,
            tc.tile_pool(name="xtp", bufs=2) as xtpool,
            tc.tile_pool(name="qk", bufs=2) as qkpool,
            tc.tile_pool(name="vp", bufs=2) as vpool,
            tc.tile_pool(name="se", bufs=3) as sepool,
            tc.tile_pool(name="yp", bufs=2) as ypool,
            tc.tile_pool(name="ytp", bufs=2) as ytpool,
            tc.tile_pool(name="op", bufs=3) as opool,
            tc.tile_pool(name="rp", bufs=4) as rpool,
            tc.tile_pool(name="ps_mm", bufs=2, space=bass.MemorySpace.PSUM) as ps_mm,
            tc.tile_pool(name="ps_att", bufs=3, space=bass.MemorySpace.PSUM) as ps_att,
        ):
          with tc.For_i(0, reps, 1) if reps > 1 else nullcontext():
            # ---- constants ----
            tri2 = singles.tile([128, 2, 128], BF16)  # keep-mask (k<=q), x2 heads
            make_upper_triangular(nc, tri2[:, 0, :], val=1.0, diag=True)
            nc.vector.tensor_copy(out=tri2[:, 1, :], in_=tri2[:, 0, :])

            wqkv = singles.tile([128, 4, 3 * D], F32)
            wqk_bf = singles.tile([128, 4, 1024], E4M3)
            wv_bf = singles.tile([128, 4, 512], BF16)
            wproj_f = singles.tile([128, 4, D], F32)
            wproj = singles.tile([128, 4, D], BF16)

            state = {}

            # ---------- stage A (loads + projections), schedulable groups ----
            def emit_load_x(b):
                x_sb = xpool.tile([128, 4, D], F32, tag="x", name="xsb")
                nc.sync.dma_start(
                    out=x_sb[:],
                    in_=x_d.ap()[b].rearrange("(q p) d -> p q d", p=128),
                )
                v_sb = vpool.tile([128, 4, H, 66], BF16, tag="v", name="vsb")
                nc.vector.memset(v_sb[:, :, :, 64:65], 1.0)
                state[b] = {"x": x_sb, "qkT": {}, "v": v_sb}
                if b == 0:
                    for half in range(2):
                        nc.scalar.dma_start(
                            out=wqkv[:, 2 * half : 2 * half + 2, 0:1024],
                            in_=wqkv_d.ap()[
                                half * 256 : half * 256 + 256, 0:1024
                            ].rearrange("(c p) o -> p c o", p=128),
                        )

            def emit_xcast(b):
                st_ = state[b]
                st_["xb"] = xbpool.tile([128, 4, D], BF16, tag="xb", name="xb")
                nc.scalar.copy(out=st_["xb"][:], in_=st_["x"][:])

            def emit_xT(b, stq):
                st_ = state[b]
                if stq == 0:
                    st_["xT"] = xtpool.tile([128, 4, D], BF16, tag="xt", name="xt")
                nc.sync.dma_start(
                    out=st_["xT"][:, :, stq * 128 : (stq + 1) * 128],
                    in_=st_["xb"][:, stq, :],
                    transpose=True,
                )

            def emit_wqk(half):
                # scaled fp8 weights; the 32*32 factor is folded into exp scale
                nc.vector.tensor_scalar_mul(
                    wqk_bf[:, 2 * half : 2 * half + 2, :],
                    wqkv[:, 2 * half : 2 * half + 2, 0:1024],
                    W8_SCALE,
                )

            def emit_xt8(b):
                st_ = state[b]
                st_["x8"] = xbpool.tile([128, 4, D], E4M3, tag="x8", name="x8")
                nc.vector.tensor_copy(out=st_["x8"][:], in_=st_["xT"][:])

            def emit_late_weights():
                nc.scalar.dma_start(
                    out=wqkv[:, :, 1024:1536],
                    in_=wqkv_d.ap()[:, 1024:1536].rearrange("(c p) o -> p c o", p=128),
                )
                nc.scalar.dma_start(
                    out=wproj_f[:],
                    in_=wproj_d.ap().rearrange("(c p) o -> p c o", p=128),
                )
                nc.vector.tensor_copy(out=wv_bf[:], in_=wqkv[:, :, 1024:1536])
                nc.scalar.copy(out=wproj[:], in_=wproj_f[:])

            def emit_qkt_group(b, mt):
                st_ = state[b]
                mp = ps_mm.tile([128, 512], F32, tag="mm", name="mp")
                for g in range(2):
                    nc.tensor.matmul(
                        mp[:],
                        wqk_bf[:, 2 * g : 2 * g + 2, mt * 128 : (mt + 1) * 128],
                        st_["x8"][:, 2 * g : 2 * g + 2, :],
                        start=(g == 0),
                        stop=(g == 1),
                        perf_mode=mybir.MatmulPerfMode.DoubleRow,
                    )
                qk = qkpool.tile([128, 512], BF16, tag=f"qk{mt}", name=f"qk{mt}")
                nc.vector.tensor_copy(out=qk[:], in_=mp[:])
                st_["qkT"][mt] = qk

            def emit_v_group(b, stt):
                st_ = state[b]
                vp_ps = ps_mm.tile([128, 512], F32, tag="mm", name="vp")
                for dc in range(4):
                    nc.tensor.matmul(
                        vp_ps[:],
                        st_["xT"][:, dc, stt * 128 : (stt + 1) * 128],
                        wv_bf[:, dc, :],
                        start=(dc == 0),
                        stop=(dc == 3),
                    )
                nc.scalar.copy(
                    out=st_["v"][:, stt, :, 0:64],
                    in_=vp_ps[:].rearrange("p (h e) -> p h e", h=H),
                )

            def proj_work(b):
                w = [partial(emit_load_x, b), partial(emit_xcast, b)]
                w += [partial(emit_xT, b, stq) for stq in range(4)]
                w.append(partial(emit_xt8, b))
                if b == 0:
                    w += [partial(emit_wqk, 0), partial(emit_wqk, 1)]
                    w.append(emit_late_weights)
                w += [partial(emit_qkt_group, b, mt) for mt in (0, 4, 1, 5, 2, 6, 3, 7)]
                w += [partial(emit_v_group, b, stt) for stt in range(4)]
                return w

            # ---------- attention ----------
            def emit_scores(b, hg):
                qkT = state[b]["qkT"]
                h0, h1 = 2 * hg, 2 * hg + 1
                se = sepool.tile([128, 4, 2, 512], BF16, tag="se", name="se")
                for kt in range(4):
                    off = 128 * kt
                    stp = ps_att.tile([128, 1024], F32, tag="att", name="stp")
                    for hi, h in enumerate((h0, h1)):
                        mtq, poq = h // 2, 64 * (h % 2)
                        mtk, pok = 4 + h // 2, 64 * (h % 2)
                        nc.tensor.matmul(
                            stp[:, hi * 512 + off : hi * 512 + 512],
                            qkT[mtk][pok : pok + 64, kt * 128 : (kt + 1) * 128],
                            qkT[mtq][poq : poq + 64, off:512],
                            start=True,
                            stop=True,
                        )
                    nc.scalar.activation(
                        out=se[:, kt, :, off:],
                        in_=stp[:].rearrange("p (hh q) -> p hh q", hh=2)[:, :, off:],
                        func=mybir.ActivationFunctionType.Exp,
                        scale=1.0 / (np.sqrt(HD) * W8_SCALE * W8_SCALE),
                    )
                    # mask the diagonal block (strict lower triangle -> 0)
                    nc.vector.tensor_mul(
                        out=se[:, kt, :, off : off + 128],
                        in0=se[:, kt, :, off : off + 128],
                        in1=tri2[:],
                    )
                return se

            def emit_y(b, hg, se, y_sb):
                st_ = state[b]
                h0, h1 = 2 * hg, 2 * hg + 1
                yp = ps_att.tile([128, 1024], F32, tag="att", name="yp")
                for hi, h in enumerate((h0, h1)):
                    for qt in range(4):
                        base = hi * 512 + qt * 65
                        for kt in range(qt + 1):
                            nc.tensor.matmul(
                                yp[:, base : base + 65],
                                se[:, kt, hi, qt * 128 : (qt + 1) * 128],
                                st_["v"][:, kt, h, 0:65],
                                start=(kt == 0),
                                stop=(kt == qt),
                            )
                rs = rpool.tile([128, 2, 4], F32, tag="rs", name="rs")
                nc.vector.reciprocal_approx_fast(
                    out=rs[:],
                    in_=yp[:].rearrange("p (hh q) -> p hh q", hh=2)[:, :, 64:260:65],
                )
                for hi, h in enumerate((h0, h1)):
                    for qt in range(4):
                        base = hi * 512 + qt * 65
                        nc.vector.tensor_scalar_mul(
                            y_sb[:, qt, 64 * h : 64 * h + 64],
                            yp[:, base : base + 64],
                            rs[:, hi, qt : qt + 1],
                        )

            def emit_yT(b, qt, y_sb, ynT):
                nc.sync.dma_start(
                    out=ynT[:, :, qt * 128 : (qt + 1) * 128],
                    in_=y_sb[:, qt, :],
                    transpose=True,
                )

            def emit_proj_group(b, qt, ynT):
                op_ps = ps_mm.tile([128, 512], F32, tag="mm", name="op")
                for dc in range(4):
                    nc.tensor.matmul(
                        op_ps[:],
                        ynT[:, dc, qt * 128 : (qt + 1) * 128],
                        wproj[:, dc, :],
                        start=(dc == 0),
                        stop=(dc == 3),
                    )
                ob = opool.tile([128, 512], F32, tag="ob", name="ob")
                nc.scalar.copy(out=ob[:], in_=op_ps[:])
                nc.sync.dma_start(
                    out=out_d.ap()[b, qt * 128 : (qt + 1) * 128, :], in_=ob[:]
                )

            # ---------- main schedule ----------
            w0 = proj_work(0)
            for f in w0[:12]:
                f()  # loads, cast, transposes, weights, qkT for head-pair 0
            se_prev = emit_scores(0, 0)
            for f in w0[12:]:
                f()
            pending_proj = []
            for b in range(B):
                queue = (proj_work(b + 1) if b + 1 < B else []) + pending_proj
                y_sb = ypool.tile([128, 4, 512], BF16, tag="y", name="ysb")
                for hg in range(4):
                    se_next = emit_scores(b, hg + 1) if hg + 1 < 4 else None
                    # fill PE while ScalarE runs the exp chain for this hg
                    for _ in range(6):
                        if queue:
                            queue.pop(0)()
                    emit_y(b, hg, se_prev, y_sb)
                    se_prev = se_next
                while queue:
                    queue.pop(0)()
                ynT = ytpool.tile([128, 4, 512], BF16, tag="yt", name="ynT")
                for qt in range(4):
                    emit_yT(b, qt, y_sb, ynT)
                # first scores of the next batch fill the normalize tail
                se_prev = emit_scores(b + 1, 0) if b + 1 < B else None
                # this batch's projection is deferred into the next attention
                pending_proj = [
                    partial(emit_proj_group, b, qt, ynT) for qt in range(4)
                ]
            for f in pending_proj:
                f()

    nc.compile()
    return nc


def kernel(x, qkv_weight, proj_weight):
    if "nc" not in _cache:
        _cache["nc"] = build_nc()
    nc = _cache["nc"]
    in_maps = [
        {
            "x": np.ascontiguousarray(x[m], dtype=np.float32),
            "wqkv": np.ascontiguousarray(qkv_weight[m], dtype=np.float32),
            "wproj": np.ascontiguousarray(proj_weight[m], dtype=np.float32),
        }
        for m in range(M)
    ]
    res = bass_utils.run_bass_kernel_spmd(nc, in_maps, core_ids=list(range(N_CORES)))
    return np.stack([res.results[m]["out"] for m in range(M)]).astype(np.float32)
